# revision 1
# baseline (speedup 1.0000x reference)
"""Trainium2 Bass kernel for graph-contrastive loss (nn_PrePrompt_75496935129282).

Computation (reference):
    self = segment_sum(logits_origin, ori_idx, G)       # [G, D]
    pos  = segment_sum(logits_pos,  pos_idx, G)         # [G, D]
    sim[g, k]  = cos(self[g], pos[k])   (eps-guarded norms)
    res[g] = log(sum_s exp(sim[g, neg_idx[g, s]])) - sim[g, g]
    out = mean(res)

Device strategy (8 NeuronCores, SPMD):
  - Nodes sharded 8 ways; within each core's shard the host orders nodes by
    graph-block (g >> 7, 16 buckets) during marshalling, so each 128-node
    chunk only targets one 128-graph block: the one-hot matmul is [128, 128]
    instead of [128, 2048] (16x less PE work). Nodes that overflow a
    bucket's fixed capacity go to a small spill section processed with
    full-width one-hots.
  - Per-core partial segment sums accumulate in PSUM as [g, d] rows, staged
    to DRAM, combined with ReduceScatter (per-core 256-graph slice) +
    AllReduce (full pos table).
  - Row-wise norms (ACT square + DVE reduce + exp(-0.5*ln(x))), normalize,
    then PE-transpose the tables to [d, g] for the cosine Gram of the
    core's 256 graphs vs all 2048.
  - numerator = rowwise dot of the local normalized self/pos slices;
    denominator = exp-accumulate of (sim + ln(count)), ln(count) being a
    host-precomputed [256, 2048] encoding of neg_idx multiplicities.
  - Per-core scalar partial losses summed on host (the unshard step).
"""

import os
import sys

sys.path.insert(0, "/opt/trn_rl_repo")

import numpy as np

import concourse.bacc as bacc
import concourse.bass as bass  # noqa: F401
import concourse.mybir as mybir
import concourse.tile as tile
from concourse import bass_isa
from concourse.bass_utils import run_bass_kernel_spmd

# Enable walrus LDWEIGHTS dedup (default-off in concourse): consecutive
# matmuls sharing the same stationary operand skip redundant weight loads.
# Verified numerically safe for this kernel's self-loading fp32r matmuls.
if os.environ.get("KERNEL_LDW_OPT", "1") == "1":
    import concourse.bass_utils as _bu

    if not getattr(_bu, "_ldw_opt_patched", False):
        _orig_run_command = _bu.run_command

        def _run_command_ldw(argv, **kw):
            argv = [
                "--enable-ldw-opt=true" if a == "--enable-ldw-opt=false" else a
                for a in argv
            ]
            return _orig_run_command(argv, **kw)

        _bu.run_command = _run_command_ldw
        _bu._ldw_opt_patched = True


def _ensure_ntff_hook():
    """The agent image's antenv lacks axon_hooks; inject it and register
    the ctypes NTFF profiling hook so trace=True works under axon."""
    import types

    import antenv

    if hasattr(antenv, "axon_hooks"):
        return
    mod = types.ModuleType("antenv.axon_hooks")
    mod._hook = None

    def set_axon_ntff_profile_hook(h):
        mod._hook = h

    def get_axon_ntff_profile_hook():
        return mod._hook

    mod.set_axon_ntff_profile_hook = set_axon_ntff_profile_hook
    mod.get_axon_ntff_profile_hook = get_axon_ntff_profile_hook
    sys.modules["antenv.axon_hooks"] = mod
    antenv.axon_hooks = mod
    try:
        from trn_agent_boot.trn_boot import _ntff_profile_via_ctypes

        mod._hook = _ntff_profile_via_ctypes("/opt/axon/libaxon_pjrt.so")
    except Exception as e:  # pragma: no cover
        print(f"ntff hook registration failed: {e}")


F32 = mybir.dt.float32
F32R = mybir.dt.float32r

G = 2048
S = 127
D = 256
NCORES = 8
P = 128
A = 4  # chunks per super-load
GLOC = G // NCORES  # 256
NBUK = 16  # graph blocks of 128

_MM_RAW = os.environ.get("KERNEL_MM_DT", "f32r")
MMDT = F32 if _MM_RAW == "f32" else F32R


def build_nc(nb_chunks: int, spill_chunks: int):
    """SPMD Bass program; per-core rows = (16*nb_chunks + spill_chunks)*128."""
    nchunk = NBUK * nb_chunks + spill_chunks
    assert nchunk % A == 0
    nsup = nchunk // A
    npad = nchunk * P

    nc = bacc.Bacc(
        "TRN2",
        target_bir_lowering=False,
        debug=False,
        num_devices=NCORES,
    )
    groups = [list(range(NCORES))]

    # ---- I/O ----
    xo = nc.dram_tensor("xo", [npad, D], MMDT, kind="ExternalInput").ap()
    io_ = nc.dram_tensor("io", [npad], F32, kind="ExternalInput").ap()
    xp = nc.dram_tensor("xp", [npad, D], MMDT, kind="ExternalInput").ap()
    ip_ = nc.dram_tensor("ip", [npad], F32, kind="ExternalInput").ap()
    lncnt = nc.dram_tensor("lncnt", [GLOC, G], F32, kind="ExternalInput").ap()
    loss_out = nc.dram_tensor("loss", [1, 1], F32, kind="ExternalOutput").ap()
    DBG = os.environ.get("KERNEL_DEBUG", "0") == "1"
    if DBG:
        dbg_pa = nc.dram_tensor("dbg_pa", [P, NBUK, D], F32, kind="ExternalOutput").ap()
        dbg_sl = nc.dram_tensor("dbg_sl", [P, 2, D], F32, kind="ExternalOutput").ap()
        dbg_pl = nc.dram_tensor("dbg_pl", [P, 2, D], F32, kind="ExternalOutput").ap()
        dbg_sim0 = nc.dram_tensor("dbg_sim0", [P, 1], F32, kind="ExternalOutput").ap()
        dbg_den = nc.dram_tensor("dbg_den", [P, 2], F32, kind="ExternalOutput").ap()
        dbg_gram = nc.dram_tensor("dbg_gram", [P, 2, G], F32, kind="ExternalOutput").ap()
        dbg_pnT = nc.dram_tensor("dbg_pnT", [P, 2, G], F32, kind="ExternalOutput").ap()
        dbg_ppart = nc.dram_tensor("dbg_ppart", [P, NBUK, D], F32, kind="ExternalOutput").ap()
        dbg_spart = nc.dram_tensor("dbg_spart", [P, NBUK, D], F32, kind="ExternalOutput").ap()

    # ---- internal DRAM for collectives (row-major [G, D] as [16, 128, D]) ----
    p_stage = nc.dram_tensor("p_stage", [NBUK, P, D], F32).ap()
    s_stage = nc.dram_tensor("s_stage", [NBUK, P, D], F32).ap()
    p_all = nc.dram_tensor("p_all", [NBUK, P, D], F32, addr_space="Shared").ap()
    p_loc = nc.dram_tensor("p_loc", [2, P, D], F32).ap()
    s_loc = nc.dram_tensor("s_loc", [2, P, D], F32).ap()

    xo_r = xo.rearrange("(s a p) d -> s p a d", a=A, p=P)
    xp_r = xp.rearrange("(s a p) d -> s p a d", a=A, p=P)
    io_r = io_.rearrange("(s a p) -> s p a", a=A, p=P)
    ip_r = ip_.rearrange("(s a p) -> s p a", a=A, p=P)

    def chunk_bucket(c):
        """-> hi block for bucket chunks, None for spill chunks."""
        return c // nb_chunks if c < NBUK * nb_chunks else None

    with tile.TileContext(nc) as tc:
        with (
            tc.tile_pool(name="const", bufs=1) as cpool,
            tc.tile_pool(name="dram", bufs=1, space="DRAM") as _dram,  # noqa: F841
        ):
            # ---- constants ----
            eps2_col = cpool.tile([P, 1], F32, tag="eps2_col")
            nc.vector.memset(eps2_col[:], 1e-16)
            iota_f = cpool.tile([P, G], F32, tag="iota_f")

            with (
                tc.tile_pool(name="stream", bufs=4) as stream,
                tc.tile_pool(name="oh", bufs=4) as ohpool,
                tc.tile_pool(name="segsb", bufs=2) as segsb,
            ):
                iota_i = ohpool.tile([P, G], mybir.dt.int32, tag="ohw")
                nc.gpsimd.iota(
                    iota_i[:], pattern=[[1, G]], base=0, channel_multiplier=0
                )
                nc.vector.tensor_copy(iota_f[:], iota_i[:])

                # ============= segment-sum phases =============
                # pos first so its (bigger) collectives overlap origin phase.
                def segment_phase(x_r, i_r, sb_tag):
                    """Bucketed one-hot matmuls -> SBUF [P, 16, D] ([g,d] rows)."""
                    with tc.tile_pool(
                        name=f"ps_seg_{sb_tag}", bufs=1, space="PSUM"
                    ) as pseg:
                        acc = pseg.tile([P, NBUK, D], F32, tag="seg")
                        for s in range(nsup):
                            xt = stream.tile([P, A, D], MMDT, tag="xt")
                            it = stream.tile([P, A], F32, tag="it")
                            nc.sync.dma_start(out=xt[:], in_=x_r[s])
                            nc.sync.dma_start(out=it[:], in_=i_r[s])
                            for a in range(A):
                                c = s * A + a
                                hi = chunk_bucket(c)
                                if hi is not None:
                                    oh = ohpool.tile([P, P], MMDT, tag="oh")
                                    nc.vector.tensor_scalar(
                                        out=oh[:],
                                        in0=iota_f[:, hi * P : (hi + 1) * P],
                                        scalar1=it[:, a : a + 1],
                                        scalar2=None,
                                        op0=mybir.AluOpType.is_equal,
                                    )
                                    nc.tensor.matmul(
                                        out=acc[:, hi, :],
                                        lhsT=oh[:],
                                        rhs=xt[:, a, :],
                                        start=(c % nb_chunks == 0),
                                        stop=False,
                                    )
                                else:
                                    # spill chunk: full-width one-hot
                                    ohw = ohpool.tile([P, G], MMDT, tag="ohw")
                                    nc.vector.tensor_scalar(
                                        out=ohw[:],
                                        in0=iota_f[:],
                                        scalar1=it[:, a : a + 1],
                                        scalar2=None,
                                        op0=mybir.AluOpType.is_equal,
                                    )
                                    for hb in range(NBUK):
                                        nc.tensor.matmul(
                                            out=acc[:, hb, :],
                                            lhsT=ohw[:, hb * P : (hb + 1) * P],
                                            rhs=xt[:, a, :],
                                            start=False,
                                            stop=(c == nchunk - 1),
                                        )
                        sb = segsb.tile([P, NBUK, D], F32, tag="sb")
                        nc.scalar.copy(sb[:], acc[:])
                        return sb

                pt_sb = segment_phase(xp_r, ip_r, "p")
                if DBG:
                    nc.sync.dma_start(out=dbg_ppart, in_=pt_sb[:])
                nc.sync.dma_start(
                    out=p_stage.rearrange("hi p d -> p hi d"),
                    in_=pt_sb[:],
                )
                nc.gpsimd.collective_compute(
                    "AllReduce",
                    mybir.AluOpType.add,
                    replica_groups=groups,
                    ins=[p_stage[:]],
                    outs=[p_all[:]],
                )
                nc.gpsimd.collective_compute(
                    "ReduceScatter",
                    mybir.AluOpType.add,
                    replica_groups=groups,
                    ins=[p_stage[:]],
                    outs=[p_loc[:]],
                )

                ot_sb = segment_phase(xo_r, io_r, "o")
                if DBG:
                    nc.sync.dma_start(out=dbg_spart, in_=ot_sb[:])
                nc.sync.dma_start(
                    out=s_stage.rearrange("hi p d -> p hi d"),
                    in_=ot_sb[:],
                )
                nc.gpsimd.collective_compute(
                    "ReduceScatter",
                    mybir.AluOpType.add,
                    replica_groups=groups,
                    ins=[s_stage[:]],
                    outs=[s_loc[:]],
                )

            from contextlib import ExitStack

            _big_ctx = ExitStack()
            bigpool = _big_ctx.enter_context(tc.tile_pool(name="big", bufs=1))

            # ================= load reduced tables ([g, d] rows) ============
            pa_sb = bigpool.tile([P, NBUK, D], F32, tag="pa")
            nc.sync.dma_start(
                out=pa_sb[:], in_=p_all.rearrange("hi p d -> p hi d")
            )
            sl_sb = bigpool.tile([P, 2, D], F32, tag="sl")
            nc.sync.dma_start(
                out=sl_sb[:], in_=s_loc.rearrange("lo p d -> p lo d")
            )
            pl_sb = bigpool.tile([P, 2, D], F32, tag="pl")
            nc.sync.dma_start(
                out=pl_sb[:], in_=p_loc.rearrange("lo p d -> p lo d")
            )

            if DBG:
                nc.sync.dma_start(out=dbg_pa, in_=pa_sb[:])
                nc.sync.dma_start(out=dbg_sl, in_=sl_sb[:])
                nc.sync.dma_start(out=dbg_pl, in_=pl_sb[:])

            # ================= norms (row-wise) =================
            def normalize_rows(src_sb, nblk, tag):
                """src [P, nblk, D] -> normalized rows (MMDT), via
                1/sqrt(rowsum(x^2)+eps2) per (partition, block)."""
                sq = bigpool.tile([P, nblk, D], F32, tag=f"sq_{tag}")
                nc.scalar.square(sq[:], src_sb[:])
                n2 = bigpool.tile([P, nblk], F32, tag=f"n2_{tag}")
                nc.vector.tensor_reduce(
                    out=n2[:],
                    in_=sq[:],
                    axis=mybir.AxisListType.X,
                    op=mybir.AluOpType.add,
                )
                lnt = bigpool.tile([P, nblk], F32, tag=f"lnt_{tag}")
                nc.scalar.activation(
                    out=lnt[:],
                    in_=n2[:],
                    func=mybir.ActivationFunctionType.Ln,
                    bias=eps2_col[:],
                )
                inv = bigpool.tile([P, nblk], F32, tag=f"inv_{tag}")
                nc.scalar.activation(
                    out=inv[:],
                    in_=lnt[:],
                    func=mybir.ActivationFunctionType.Exp,
                    scale=-0.5,
                )
                out = bigpool.tile([P, nblk, D], MMDT, tag=f"nrm_{tag}")
                for b in range(nblk):
                    nc.vector.tensor_scalar(
                        out=out[:, b, :],
                        in0=src_sb[:, b, :],
                        scalar1=inv[:, b : b + 1],
                        scalar2=None,
                        op0=mybir.AluOpType.mult,
                    )
                return out

            pn = normalize_rows(pa_sb, NBUK, "nb")  # pos-hat rows, all graphs
            sn = normalize_rows(sl_sb, 2, "na")  # self-hat rows, local
            pnl = normalize_rows(pl_sb, 2, "nbl")  # pos-hat rows, local

            # numerator: sim0 = rowwise dot of local normalized tables
            s0tmp = bigpool.tile([P, 2, D], F32, tag="s0tmp")
            nc.vector.tensor_tensor(
                out=s0tmp[:], in0=sn[:], in1=pnl[:], op=mybir.AluOpType.mult
            )
            sim0 = bigpool.tile([P, 1], F32, tag="sim0")
            nc.vector.tensor_reduce(
                out=sim0[:],
                in_=s0tmp[:],
                axis=mybir.AxisListType.XY,
                op=mybir.AluOpType.add,
            )

            if DBG:
                nc.sync.dma_start(out=dbg_sim0, in_=sim0[:])

            # ================= transpose to [d, g] for the Gram =============
            from concourse.masks import make_identity

            ident_f = bigpool.tile([P, P], F32, tag="ident_f")
            make_identity(nc, ident_f[:])
            if MMDT is F32:
                ident = ident_f
            else:
                ident = bigpool.tile([P, P], MMDT, tag="ident")
                nc.vector.tensor_copy(ident[:], ident_f[:])

            pn_T = bigpool.tile([P, 2, G], MMDT, tag="pnT")
            sn_T = bigpool.tile([P, 2, GLOC], MMDT, tag="snT")
            with tc.tile_pool(name="ps_tr", bufs=4, space="PSUM") as ptr:
                for hi in range(NBUK):
                    for db in range(2):
                        tps = ptr.tile([P, P], MMDT, tag="tr")
                        nc.tensor.transpose(
                            out=tps[:],
                            in_=pn[:, hi, db * P : (db + 1) * P],
                            identity=ident[:],
                        )
                        nc.vector.tensor_copy(
                            pn_T[:, db, hi * P : (hi + 1) * P], tps[:]
                        )
                for lo in range(2):
                    for db in range(2):
                        tps = ptr.tile([P, P], MMDT, tag="tr")
                        nc.tensor.transpose(
                            out=tps[:],
                            in_=sn[:, lo, db * P : (db + 1) * P],
                            identity=ident[:],
                        )
                        nc.vector.tensor_copy(
                            sn_T[:, db, lo * P : (lo + 1) * P], tps[:]
                        )

            if DBG:
                pnT_f = bigpool.tile([P, 2, G], F32, tag="pnT_f")
                nc.vector.tensor_copy(pnT_f[:], pn_T[:])
                nc.sync.dma_start(out=dbg_pnT, in_=pnT_f[:])

            # lncnt for the two local 128-graph blocks
            lnc_sb = bigpool.tile([P, 2, G], F32, tag="lnc")
            nc.sync.dma_start(
                out=lnc_sb[:],
                in_=lncnt.rearrange("(lo p) g -> p lo g", lo=2, p=P),
            )

            # ================= Gram + loss =================
            lden = [None, None]
            with tc.tile_pool(name="ps_gram", bufs=1, space="PSUM") as pgram:
                for lo in range(2):
                    gram = pgram.tile([P, G], F32, tag="gram")
                    for db in range(2):
                        lhsT = sn_T[:, db, lo * P : (lo + 1) * P]
                        for nb in range(4):
                            nc.tensor.matmul(
                                out=gram[:, nb * 512 : (nb + 1) * 512],
                                lhsT=lhsT,
                                rhs=pn_T[:, db, nb * 512 : (nb + 1) * 512],
                                start=(db == 0),
                                stop=(db == 1),
                            )
                    if DBG:
                        gsb = bigpool.tile([P, G], F32, tag="gsb")
                        nc.vector.tensor_copy(gsb[:], gram[:])
                        nc.sync.dma_start(out=dbg_gram[:, lo, :], in_=gsb[:])
                    simln = bigpool.tile([P, G], F32, tag="simln")
                    nc.vector.tensor_tensor(
                        out=simln[:],
                        in0=gram[:],
                        in1=lnc_sb[:, lo, :],
                        op=mybir.AluOpType.add,
                    )
                    escr = bigpool.tile([P, G], F32, tag="escr")
                    den = bigpool.tile([P, 1], F32, tag=f"den{lo}")
                    nc.scalar.activation(
                        out=escr[:],
                        in_=simln[:],
                        func=mybir.ActivationFunctionType.Exp,
                        accum_out=den[:],
                    )
                    if DBG:
                        nc.sync.dma_start(out=dbg_den[:, lo : lo + 1], in_=den[:])
                    ld = bigpool.tile([P, 1], F32, tag=f"lden{lo}")
                    nc.scalar.activation(
                        out=ld[:],
                        in_=den[:],
                        func=mybir.ActivationFunctionType.Ln,
                    )
                    lden[lo] = ld

            # loss_col = lden0 + lden1 - sim0 ; partition-sum -> scalar
            t0 = bigpool.tile([P, 1], F32, tag="t0")
            nc.vector.tensor_tensor(
                out=t0[:], in0=lden[0][:], in1=lden[1][:], op=mybir.AluOpType.add
            )
            t1 = bigpool.tile([P, 1], F32, tag="t1")
            nc.vector.tensor_tensor(
                out=t1[:], in0=t0[:], in1=sim0[:], op=mybir.AluOpType.subtract
            )
            lsum = bigpool.tile([P, 1], F32, tag="lsum")
            nc.gpsimd.partition_all_reduce(
                lsum[:], t1[:], P, bass_isa.ReduceOp.add
            )
            nc.sync.dma_start(out=loss_out[:], in_=lsum[:1, :])

            _big_ctx.close()
    nc.compile()
    return nc


def _bucketize(gids, nb_chunks, spill_cap):
    """Order node positions so each 128-row chunk targets one graph block.

    Returns (src, dst): node position src[i] goes to padded row dst[i];
    uncovered rows are padding. Raises if spill overflows."""
    cap = nb_chunks * P
    key = (gids >> 7).astype(np.int64)
    order = np.argsort(key, kind="stable")
    counts = np.bincount(key, minlength=NBUK)
    starts = np.zeros(NBUK + 1, np.int64)
    np.cumsum(counts, out=starts[1:])
    src = []
    dst = []
    spill = []
    for b in range(NBUK):
        seg = order[starts[b] : starts[b + 1]]
        keep = seg[:cap]
        src.append(keep)
        dst.append(b * cap + np.arange(len(keep)))
        if len(seg) > cap:
            spill.append(seg[cap:])
    if spill:
        sp = np.concatenate(spill)
        if len(sp) > spill_cap:
            raise RuntimeError(f"bucket spill overflow: {len(sp)} > {spill_cap}")
        src.append(sp)
        dst.append(NBUK * cap + np.arange(len(sp)))
    return np.concatenate(src), np.concatenate(dst)


def _prep_inputs(logits_origin, logits_pos, ori_idx, pos_idx, neg_idx, nb_chunks,
                 spill_chunks):
    xo = np.ascontiguousarray(np.asarray(logits_origin, dtype=np.float32))
    xp = np.ascontiguousarray(np.asarray(logits_pos, dtype=np.float32))
    oi = np.asarray(ori_idx).astype(np.int64)
    pi = np.asarray(pos_idx).astype(np.int64)
    neg = np.asarray(neg_idx)
    n = xo.shape[0]
    assert xo.shape == (n, D) and xp.shape == (n, D)
    assert neg.shape == (G, S)

    nloc = (n + NCORES - 1) // NCORES
    npad = (NBUK * nb_chunks + spill_chunks) * P
    spill_cap = spill_chunks * P

    cnt = np.zeros((G, G), dtype=np.float64)
    rows = np.repeat(np.arange(G), S)
    np.add.at(cnt, (rows, neg.ravel().astype(np.int64)), 1.0)
    with np.errstate(divide="ignore"):
        lncnt = np.where(cnt > 0, np.log(cnt), -30000.0).astype(np.float32)

    in_maps = []
    for r in range(NCORES):
        lo = r * nloc
        hi = min(n, lo + nloc)
        xo_r = np.zeros((npad, D), np.float32)
        xp_r = np.zeros((npad, D), np.float32)
        io_r = np.full((npad,), -1.0, np.float32)
        ip_r = np.full((npad,), -1.0, np.float32)
        for x_full, g_shard, x_dst, g_dst in (
            (xo[lo:hi], oi[lo:hi], xo_r, io_r),
            (xp[lo:hi], pi[lo:hi], xp_r, ip_r),
        ):
            src, dst = _bucketize(g_shard, nb_chunks, spill_cap)
            x_dst[dst] = x_full[src]
            g_dst[dst] = g_shard[src].astype(np.float32)
        in_maps.append(
            {
                "xo": xo_r,
                "io": io_r,
                "xp": xp_r,
                "ip": ip_r,
                "lncnt": np.ascontiguousarray(lncnt[r * GLOC : (r + 1) * GLOC]),
            }
        )
    return in_maps


def kernel(
    logits_origin,
    logits_pos,
    ori_idx,
    pos_idx,
    neg_idx,
    _trace=False,
    _tmpdir=None,
):
    n = np.asarray(logits_origin).shape[0]
    nloc = (n + NCORES - 1) // NCORES
    # bucket capacity: mean + 2.5 sigma (Poisson), whole chunks; rare
    # overflow lands in the spill section (cap 512, overflow prob ~0)
    mean_b = nloc / NBUK
    cap = mean_b + 2.5 * np.sqrt(mean_b)
    nb_chunks = max(1, int(np.ceil(cap / P)))
    spill_chunks = 4
    while (NBUK * nb_chunks + spill_chunks) % A != 0:
        spill_chunks += 1

    in_maps = _prep_inputs(
        logits_origin, logits_pos, ori_idx, pos_idx, neg_idx, nb_chunks,
        spill_chunks,
    )
    if _trace:
        _ensure_ntff_hook()
    nc = build_nc(nb_chunks, spill_chunks)
    res = run_bass_kernel_spmd(
        nc,
        in_maps,
        core_ids=list(range(NCORES)),
        trace=_trace,
        tmpdir=_tmpdir,
    )
    kernel._last_results = res
    total = sum(float(res.results[r]["loss"][0, 0]) for r in range(NCORES))
    return np.asarray(np.float32(total / G))


kernel._last_results = None


if __name__ == "__main__":
    rng = np.random.default_rng(0)
    n = 4096
    inputs = {
        "logits_origin": rng.standard_normal((n, D), dtype=np.float32),
        "logits_pos": rng.standard_normal((n, D), dtype=np.float32),
        "ori_idx": rng.integers(0, G, n, dtype=np.int32),
        "pos_idx": rng.integers(0, G, n, dtype=np.int32),
        "neg_idx": rng.integers(0, G, (G, S), dtype=np.int32),
    }

    def np_ref(logits_origin, logits_pos, ori_idx, pos_idx, neg_idx):
        x = logits_origin.astype(np.float64)
        y = logits_pos.astype(np.float64)
        self_l = np.zeros((G, D))
        pos_l = np.zeros((G, D))
        np.add.at(self_l, ori_idx, x)
        np.add.at(pos_l, pos_idx, y)
        eps = 1e-8
        na = np.maximum(np.linalg.norm(self_l, axis=1), eps)
        nb = np.maximum(np.linalg.norm(pos_l, axis=1), eps)
        sh = self_l / na[:, None]
        ph = pos_l / nb[:, None]
        gram = sh @ ph.T
        sim0 = np.einsum("gd,gd->g", sh, ph)
        e = np.exp(gram)
        den = np.array([e[g, neg_idx[g]].sum() for g in range(G)])
        res = np.log(den) - sim0
        return res.mean()

    expected = np_ref(**inputs)
    actual = kernel(**inputs)
    err = abs(actual - expected) / max(abs(expected), 1e-12)
    print(f"expected={expected:.6f} actual={float(actual):.6f} relerr={err:.3e}")



# revision 9
# speedup vs baseline: 2.1731x; 2.1731x over previous
"""Trainium2 Bass kernel for graph-contrastive loss (nn_PrePrompt_75496935129282).

Computation (reference):
    self = segment_sum(logits_origin, ori_idx, G)       # [G, D]
    pos  = segment_sum(logits_pos,  pos_idx, G)         # [G, D]
    sim[g, k]  = cos(self[g], pos[k])   (eps-guarded norms)
    res[g] = log(sum_s exp(sim[g, neg_idx[g, s]])) - sim[g, g]
    out = mean(res)

Device strategy (8 NeuronCores, SPMD, fp16 datapath):
  - Nodes sharded 8 ways with GLOBAL bucket balancing: host sorts all nodes
    by graph block (g >> 7, 16 buckets) and deals each bucket evenly across
    cores, so every core has identical per-bucket chunk counts and there is
    no spill section. Each 128-node chunk targets one 128-graph block:
    one-hot (DVE is_equal vs iota, fp16) matmul accumulates [128, 256]
    per-bucket partials in a rotating PSUM tile.
  - Partials are cast to fp16 and staged to DRAM per bucket as soon as the
    bucket's 13 chunks finish; ReduceScatters are split in halves (buckets
    0-7 / 8-15) so the first half's collective overlaps the rest of the
    phase. Core r owns global graph blocks {r, r+8}.
  - pos chain: RS -> normalize 256 local rows -> PE-transpose to [d, g] ->
    AllGather (fp16) -> assemble full [d, 2048] pos-hat table. All of it
    overlaps the origin segment-sum phase.
  - self rows stay UNnormalized: 1/|s_g| folds into the exp row scale.
    Gram = snT^T @ pn_T (fp16, f32 PSUM), + ln(count) (host-precomputed
    neg_idx multiplicities, fp16), exp with row scale + accum -> denominator.
  - Device ships den / sim0_raw / inv-self-norm per local graph; host does
    loss = mean(log(den) - sim0_raw * invna) over all 2048 graphs.
"""

import os
import sys

sys.path.insert(0, "/opt/trn_rl_repo")

import numpy as np

import concourse.bacc as bacc
import concourse.bass as bass  # noqa: F401
import concourse.mybir as mybir
import concourse.tile as tile
from concourse import bass_isa  # noqa: F401
from concourse.bass_utils import run_bass_kernel_spmd

# Enable walrus LDWEIGHTS dedup (default-off in concourse): consecutive
# matmuls sharing the same stationary operand skip redundant weight loads.
if os.environ.get("KERNEL_LDW_OPT", "1") == "1":
    import concourse.bass_utils as _bu

    if not getattr(_bu, "_ldw_opt_patched", False):
        _orig_run_command = _bu.run_command

        def _run_command_ldw(argv, **kw):
            argv = [
                "--enable-ldw-opt=true" if a == "--enable-ldw-opt=false" else a
                for a in argv
            ]
            return _orig_run_command(argv, **kw)

        _bu.run_command = _run_command_ldw
        _bu._ldw_opt_patched = True


def _ensure_ntff_hook():
    """The agent image's antenv lacks axon_hooks; inject it and register
    the ctypes NTFF profiling hook so trace=True works under axon."""
    import types

    import antenv

    if hasattr(antenv, "axon_hooks"):
        return
    mod = types.ModuleType("antenv.axon_hooks")
    mod._hook = None

    def set_axon_ntff_profile_hook(h):
        mod._hook = h

    def get_axon_ntff_profile_hook():
        return mod._hook

    mod.set_axon_ntff_profile_hook = set_axon_ntff_profile_hook
    mod.get_axon_ntff_profile_hook = get_axon_ntff_profile_hook
    sys.modules["antenv.axon_hooks"] = mod
    antenv.axon_hooks = mod
    try:
        from trn_agent_boot.trn_boot import _ntff_profile_via_ctypes

        mod._hook = _ntff_profile_via_ctypes("/opt/axon/libaxon_pjrt.so")
    except Exception as e:  # pragma: no cover
        print(f"ntff hook registration failed: {e}")


F32 = mybir.dt.float32
F16 = mybir.dt.float16

G = 2048
S = 127
D = 256
NCORES = 8
P = 128
A = 4  # chunks per super-load
NBUK = 16  # graph blocks of 128
JL = 2  # local graph blocks per core (core r owns blocks r and r+8)
GLOC = JL * P


def build_nc(nb_chunks: int):
    """SPMD Bass program; per-core rows = 16*nb_chunks*128, zero spill."""
    nchunk = NBUK * nb_chunks
    assert nchunk % A == 0
    nsup = nchunk // A

    nc = bacc.Bacc(
        "TRN2",
        target_bir_lowering=False,
        debug=False,
        num_devices=NCORES,
    )
    groups = [list(range(NCORES))]

    # ---- I/O ----
    xo = nc.dram_tensor("xo", [nsup, P, A, D], F16, kind="ExternalInput").ap()
    io_ = nc.dram_tensor("io", [P, nchunk], F32, kind="ExternalInput").ap()
    xp = nc.dram_tensor("xp", [nsup, P, A, D], F16, kind="ExternalInput").ap()
    ip_ = nc.dram_tensor("ip", [P, nchunk], F32, kind="ExternalInput").ap()
    lncnt = nc.dram_tensor("lncnt", [JL, P, G], F16, kind="ExternalInput").ap()
    out_d = nc.dram_tensor("out", [P, 6], F32, kind="ExternalOutput").ap()

    # ---- internal DRAM for collectives ----
    p_stage = nc.dram_tensor("p_stage", [NBUK, P, D], F16).ap()
    s_stage = nc.dram_tensor("s_stage", [NBUK, P, D], F16).ap()
    p_loc = nc.dram_tensor("p_loc", [JL, P, D], F16).ap()
    s_loc = nc.dram_tensor("s_loc", [JL, P, D], F16).ap()
    ag_in = nc.dram_tensor("ag_in", [P, 2, JL, P], F16).ap()  # [d, db, j, p]
    ag_all = nc.dram_tensor(
        "ag_all", [NCORES, P, 2, JL, P], F16, addr_space="Shared"
    ).ap()

    with tile.TileContext(nc) as tc:
        with (
            tc.tile_pool(name="const", bufs=1) as cpool,
            tc.tile_pool(name="big", bufs=1) as big,
            tc.tile_pool(name="stream", bufs=6) as stream,
            tc.tile_pool(name="oh", bufs=8) as ohpool,
            tc.tile_pool(name="segsb", bufs=4) as segsb,
            tc.tile_pool(name="ps_seg", bufs=3, space="PSUM") as pseg,
            tc.tile_pool(name="ps_tr", bufs=2, space="PSUM") as ptr,
            tc.tile_pool(name="ps_gram", bufs=2, space="PSUM") as pgram,
        ):
            # ---- constants ----
            eps_col = cpool.tile([P, 1], F32, tag="eps_col")
            nc.vector.memset(eps_col[:], 1e-16)
            iota_i = cpool.tile([P, G], mybir.dt.int32, tag="iota_i")
            nc.gpsimd.iota(iota_i[:], pattern=[[1, G]], base=0, channel_multiplier=0)
            iota_f = cpool.tile([P, G], F16, tag="iota_f")
            nc.vector.tensor_copy(iota_f[:], iota_i[:])

            from concourse.masks import make_identity

            ident_f = cpool.tile([P, P], F32, tag="ident_f")
            make_identity(nc, ident_f[:])
            ident = cpool.tile([P, P], F16, tag="ident")
            nc.vector.tensor_copy(ident[:], ident_f[:])

            # lncnt loaded during phase 1 (ACT-issued DMA; ACT idle then)
            lnc_sb = big.tile([P, JL, G], F16, tag="lnc")
            nc.scalar.dma_start(
                out=lnc_sb[:], in_=lncnt.rearrange("j p g -> p j g")
            )

            # index tables: one DMA each, whole phase
            it_p = big.tile([P, nchunk], F32, tag="it_p")
            nc.sync.dma_start(out=it_p[:], in_=ip_)
            it_o = big.tile([P, nchunk], F32, tag="it_o")
            nc.sync.dma_start(out=it_o[:], in_=io_)

            # persistent tail tiles
            pl = big.tile([P, JL, D], F16, tag="pl")  # local pos rows (raw)
            sl = big.tile([P, JL, D], F16, tag="sl")  # local self rows (raw)
            pnl = big.tile([P, JL, D], F16, tag="pnl")  # normalized local pos
            pnlT = big.tile([P, 2, JL, P], F16, tag="pnlT")  # [d, db, j, p]
            snT = big.tile([P, 2, JL, P], F16, tag="snT")  # [d, db, j, p]
            pn_T = big.tile([P, 2, G], F16, tag="pn_T")  # [d, db, g-cols]
            n2p = big.tile([P, JL], F32, tag="n2p")
            nap = big.tile([P, JL], F32, tag="nap")
            invp = big.tile([P, JL], F32, tag="invp")
            n2s = big.tile([P, JL], F32, tag="n2s")
            nas = big.tile([P, JL], F32, tag="nas")
            invs = big.tile([P, JL], F32, tag="invs")
            scr = big.tile([P, D], F16, tag="scr")  # square scratch
            escr = big.tile([P, 512], F16, tag="escr")  # exp scratch
            dacc = big.tile([P, JL, 4], F32, tag="dacc")  # den quarters
            s0t = big.tile([P, D], F32, tag="s0t")  # sim0 scratch
            out_sb = big.tile([P, 6], F32, tag="out_sb")

            def emit_norm_inv(src, j, n2, na, inv):
                """inv[:, j] = 1/sqrt(sum_d src[:, j, :]^2 + 1e-16)."""
                nc.scalar.activation(
                    out=scr[:],
                    in_=src[:, j, :],
                    func=mybir.ActivationFunctionType.Square,
                    accum_out=n2[:, j : j + 1],
                )
                nc.scalar.activation(
                    out=na[:, j : j + 1],
                    in_=n2[:, j : j + 1],
                    func=mybir.ActivationFunctionType.Sqrt,
                    bias=eps_col[:],
                )
                nc.vector.reciprocal(inv[:, j : j + 1], na[:, j : j + 1])

            def emit_transpose_pair(src, j, dst):
                """dst[d, db, j, p] = src[p, j, db*128+d] for db in 0,1."""
                for db in range(2):
                    tps = ptr.tile([P, P], F16, tag="tr")
                    nc.tensor.transpose(
                        out=tps[:],
                        in_=src[:, j, db * P : (db + 1) * P],
                        identity=ident[:],
                    )
                    nc.vector.tensor_copy(dst[:, db, j, :], tps[:])

            def emit_gram_pair(j, h0):
                """Two PSUM tiles [P, 512] = snT_j^T @ pn_T col-blocks h0, h0+1.

                db-outer order so walrus dedups the shared LDWEIGHTS."""
                g0 = pgram.tile([P, 512], F32, tag="gram")
                g1 = pgram.tile([P, 512], F32, tag="gram")
                for db in range(2):
                    for h, gt in ((h0, g0), (h0 + 1, g1)):
                        nc.tensor.matmul(
                            out=gt[:],
                            lhsT=snT[:, db, j, :],
                            rhs=pn_T[:, db, h * 512 : (h + 1) * 512],
                            start=(db == 0),
                            stop=(db == 1),
                        )
                return g0, g1

            # ============= segment-sum phases =============
            def segment_phase(x_r, it_all, stage, tag, hooks):
                for s in range(nsup):
                    xt = stream.tile([P, A, D], F16, tag="xt")
                    nc.sync.dma_start(out=xt[:], in_=x_r[s])
                    for a in range(A):
                        c = s * A + a
                        b = c // nb_chunks
                        k = c % nb_chunks
                        if k == 0:
                            acc = pseg.tile([P, D], F32, tag="acc")
                            segment_phase.acc = acc
                        acc = segment_phase.acc
                        oh = ohpool.tile([P, P], F16, tag="oh")
                        nc.vector.tensor_scalar(
                            out=oh[:],
                            in0=iota_f[:, b * P : (b + 1) * P],
                            scalar1=it_all[:, c : c + 1],
                            scalar2=None,
                            op0=mybir.AluOpType.is_equal,
                        )
                        nc.tensor.matmul(
                            out=acc[:],
                            lhsT=oh[:],
                            rhs=xt[:, a, :],
                            start=(k == 0),
                            stop=(k == nb_chunks - 1),
                        )
                        if k == nb_chunks - 1:
                            sb = segsb.tile([P, D], F16, tag="sb" + tag)
                            nc.scalar.copy(sb[:], acc[:])
                            nc.scalar.dma_start(out=stage[b], in_=sb[:])
                            for h in hooks.get(b, []):
                                h()

            # ---- phase 1 (pos) hooks ----
            def trig_rs(stage, loc, half):
                def f():
                    nc.gpsimd.collective_compute(
                        "ReduceScatter",
                        mybir.AluOpType.add,
                        replica_groups=groups,
                        ins=[stage[half * 8 : (half + 1) * 8]],
                        outs=[loc[half : half + 1]],
                    )

                return f

            segment_phase(
                xp,
                it_p,
                p_stage,
                "p",
                {
                    7: [trig_rs(p_stage, p_loc, 0)],
                    NBUK - 1: [trig_rs(p_stage, p_loc, 1)],
                },
            )

            # ---- phase 2 (origin) hooks: pos chain + s collectives ----
            def p_chain_norms():
                # p_loc load on gpsimd queue (after RS triggers; waits RS done)
                nc.gpsimd.dma_start(
                    out=pl[:], in_=p_loc.rearrange("j p d -> p j d")
                )
                for j in range(JL):
                    emit_norm_inv(pl, j, n2p, nap, invp)
                for j in range(JL):
                    nc.vector.tensor_scalar(
                        out=pnl[:, j, :],
                        in0=pl[:, j, :],
                        scalar1=invp[:, j : j + 1],
                        scalar2=None,
                        op0=mybir.AluOpType.mult,
                    )

            def p_chain_ag():
                for j in range(JL):
                    emit_transpose_pair(pnl, j, pnlT)
                nc.scalar.dma_start(out=ag_in, in_=pnlT[:])
                nc.gpsimd.collective_compute(
                    "AllGather",
                    mybir.AluOpType.bypass,
                    replica_groups=groups,
                    ins=[ag_in[:]],
                    outs=[ag_all[:]],
                )

            def s0_chain():
                # local self rows, first half (j=0): norms + transpose
                nc.scalar.dma_start(out=sl[:, 0, :], in_=s_loc[0])
                emit_norm_inv(sl, 0, n2s, nas, invs)
                emit_transpose_pair(sl, 0, snT)

            segment_phase(
                xo,
                it_o,
                s_stage,
                "s",
                {
                    5: [p_chain_norms],
                    7: [p_chain_ag, trig_rs(s_stage, s_loc, 0)],
                    13: [s0_chain],
                    NBUK - 1: [trig_rs(s_stage, s_loc, 1)],
                },
            )

            # ================= tail =================
            # pn_T assembly: 16 DMAs [d, j, p] -> pn_T[:, db, r*256:(r+1)*256]
            for r in range(NCORES):
                for db in range(2):
                    nc.scalar.dma_start(
                        out=pn_T[:, db, r * GLOC : (r + 1) * GLOC],
                        in_=ag_all[r, :, db],
                    )

            # DVE: fold invna + lncnt into gram (in place), then exp+accum
            def emit_stt_exp(j, h, gt):
                nc.vector.scalar_tensor_tensor(
                    out=gt[:],
                    in0=gt[:],
                    scalar=invs[:, j : j + 1],
                    in1=lnc_sb[:, j, h * 512 : (h + 1) * 512],
                    op0=mybir.AluOpType.mult,
                    op1=mybir.AluOpType.add,
                )
                nc.scalar.activation(
                    out=escr[:],
                    in_=gt[:],
                    func=mybir.ActivationFunctionType.Exp,
                    accum_out=dacc[:, j, h : h + 1],
                )

            def emit_den(j):
                nc.vector.tensor_tensor(
                    out=dacc[:, j, 0:1],
                    in0=dacc[:, j, 0:1],
                    in1=dacc[:, j, 1:2],
                    op=mybir.AluOpType.add,
                )
                nc.vector.tensor_tensor(
                    out=dacc[:, j, 2:3],
                    in0=dacc[:, j, 2:3],
                    in1=dacc[:, j, 3:4],
                    op=mybir.AluOpType.add,
                )
                nc.vector.tensor_tensor(
                    out=out_sb[:, j : j + 1],
                    in0=dacc[:, j, 0:1],
                    in1=dacc[:, j, 2:3],
                    op=mybir.AluOpType.add,
                )

            # sim0 raw (sl raw . pnl normalized)
            def emit_sim0(j):
                nc.vector.tensor_tensor(
                    out=s0t[:],
                    in0=sl[:, j, :],
                    in1=pnl[:, j, :],
                    op=mybir.AluOpType.mult,
                )
                nc.vector.tensor_reduce(
                    out=out_sb[:, 2 + j : 3 + j],
                    in_=s0t[:],
                    axis=mybir.AxisListType.X,
                    op=mybir.AluOpType.add,
                )

            def emit_contrastive(j):
                for h0 in (0, 2):
                    g0, g1 = emit_gram_pair(j, h0)
                    emit_stt_exp(j, h0, g0)
                    emit_stt_exp(j, h0 + 1, g1)
                emit_den(j)
                emit_sim0(j)
                nc.vector.tensor_copy(out_sb[:, 4 + j : 5 + j], invs[:, j : j + 1])

            # j=0: snT/invs ready from the in-phase hook
            emit_contrastive(0)

            # j=1: waits on s RS#2
            nc.scalar.dma_start(out=sl[:, 1, :], in_=s_loc[1])
            emit_norm_inv(sl, 1, n2s, nas, invs)
            emit_transpose_pair(sl, 1, snT)
            emit_contrastive(1)

            nc.scalar.dma_start(out=out_d, in_=out_sb[:])

    nc.compile()
    return nc


def _marshal_shard(x, gids, nb_chunks):
    """Per-core marshalling: bucket-ordered rows + fp16 x stream layout.

    Returns (x_dev [nsup, P, A, D] f16, it_dev [P, nchunk] f16)."""
    nchunk = NBUK * nb_chunks
    cap = nb_chunks * P
    rows = nchunk * P
    x_lin = np.zeros((rows, D), np.float16)
    i_lin = np.full((rows,), -1.0, np.float32)
    key = (gids >> 7).astype(np.int64)
    order = np.argsort(key, kind="stable")
    counts = np.bincount(key, minlength=NBUK)
    starts = np.zeros(NBUK + 1, np.int64)
    np.cumsum(counts, out=starts[1:])
    for b in range(NBUK):
        seg = order[starts[b] : starts[b + 1]]
        assert len(seg) <= cap, f"bucket {b} overflow: {len(seg)} > {cap}"
        x_lin[b * cap : b * cap + len(seg)] = x[seg]
        i_lin[b * cap : b * cap + len(seg)] = gids[seg].astype(np.float32)
    nsup = nchunk // A
    x_dev = np.ascontiguousarray(
        x_lin.reshape(nsup, A, P, D).transpose(0, 2, 1, 3)
    )
    it_dev = np.ascontiguousarray(i_lin.reshape(nchunk, P).T)
    return x_dev, it_dev


def _prep_inputs(logits_origin, logits_pos, ori_idx, pos_idx, neg_idx):
    xo = np.asarray(logits_origin, dtype=np.float16)
    xp = np.asarray(logits_pos, dtype=np.float16)
    oi = np.asarray(ori_idx).astype(np.int64)
    pi = np.asarray(pos_idx).astype(np.int64)
    neg = np.asarray(neg_idx)
    n = xo.shape[0]
    assert xo.shape == (n, D) and xp.shape == (n, D)
    assert neg.shape == (G, S)

    # global balance: deal each bucket's nodes evenly across cores
    def assign(gids):
        key = (gids >> 7).astype(np.int64)
        core_of = np.empty(n, np.int64)
        maxpc = 0
        for b in range(NBUK):
            pos_b = np.flatnonzero(key == b)
            parts = np.array_split(pos_b, NCORES)
            maxpc = max(maxpc, max(len(p) for p in parts))
            for r, part in enumerate(parts):
                core_of[part] = r
        return core_of, maxpc

    core_o, max_o = assign(oi)
    core_p, max_p = assign(pi)
    nb_chunks = max(1, -(-max(max_o, max_p) // P))
    if (NBUK * nb_chunks) % A != 0:
        nb_chunks += 1  # 16*nb is %4==0 always; defensive

    # ln(count) table with AG column permutation
    cnt = np.zeros((G, G), dtype=np.float64)
    rows = np.repeat(np.arange(G), S)
    np.add.at(cnt, (rows, neg.ravel().astype(np.int64)), 1.0)
    with np.errstate(divide="ignore"):
        lncnt = np.where(cnt > 0, np.log(cnt), -30000.0).astype(np.float32)
    c = np.arange(G)
    gmap = (c // GLOC + 8 * ((c % GLOC) // P)) * P + (c % P)

    in_maps = []
    for r in range(NCORES):
        xo_dev, io_dev = _marshal_shard(xo[core_o == r], oi[core_o == r], nb_chunks)
        xp_dev, ip_dev = _marshal_shard(xp[core_p == r], pi[core_p == r], nb_chunks)
        lnc_dev = np.empty((JL, P, G), np.float16)
        for j in range(JL):
            gb = r + 8 * j
            lnc_dev[j] = lncnt[gb * P : (gb + 1) * P][:, gmap]
        in_maps.append(
            {
                "xo": xo_dev,
                "io": io_dev,
                "xp": xp_dev,
                "ip": ip_dev,
                "lncnt": lnc_dev,
            }
        )
    return in_maps, nb_chunks


def kernel(
    logits_origin,
    logits_pos,
    ori_idx,
    pos_idx,
    neg_idx,
    _trace=False,
    _tmpdir=None,
):
    in_maps, nb_chunks = _prep_inputs(
        logits_origin, logits_pos, ori_idx, pos_idx, neg_idx
    )
    if _trace:
        _ensure_ntff_hook()
    nc = build_nc(nb_chunks)
    res = run_bass_kernel_spmd(
        nc,
        in_maps,
        core_ids=list(range(NCORES)),
        trace=_trace,
        tmpdir=_tmpdir,
    )
    kernel._last_results = res
    total = 0.0
    for r in range(NCORES):
        o = np.asarray(res.results[r]["out"], dtype=np.float64)
        den = o[:, 0:2]
        s0r = o[:, 2:4]
        inv = o[:, 4:6]
        total += float(np.sum(np.log(den) - s0r * inv))
    return np.asarray(np.float32(total / G))


kernel._last_results = None


def _numpy_emulate(logits_origin, logits_pos, ori_idx, pos_idx, neg_idx):
    """Pure-numpy emulation of the device algorithm (fp16 quantization of
    inputs only) for fast host-logic validation of bucketing/permutation."""
    in_maps, nb_chunks = _prep_inputs(
        logits_origin, logits_pos, ori_idx, pos_idx, neg_idx
    )
    nchunk = NBUK * nb_chunks
    nsup = nchunk // A

    def segsum(x_dev, it_dev):
        x_lin = x_dev.transpose(0, 2, 1, 3).reshape(nchunk * P, D).astype(np.float64)
        i_lin = it_dev.T.reshape(nchunk * P).astype(np.int64)
        out = np.zeros((NBUK, P, D))
        for b in range(NBUK):
            rows = slice(b * nb_chunks * P, (b + 1) * nb_chunks * P)
            gl = i_lin[rows] - b * P
            ok = gl >= 0
            np.add.at(out[b], gl[ok], x_lin[rows][ok])
        return out  # [16, 128, D] partials, fp16-quantized

    stage_p = [segsum(m["xp"], m["ip"]) for m in in_maps]
    stage_s = [segsum(m["xo"], m["io"]) for m in in_maps]
    # split-RS: core r gets blocks {r, 8+j*?}: half0 -> block r, half1 -> 8+r
    total = 0.0
    # full tables for AG emulation
    pos_full = np.sum(stage_p, axis=0)  # [16, 128, D]
    pn_full = pos_full.reshape(G, D)
    invp_full = 1.0 / np.sqrt(np.sum(pn_full**2, axis=1) + 1e-16)
    pn_hat = pn_full * invp_full[:, None]
    c = np.arange(G)
    gmap = (c // GLOC + 8 * ((c % GLOC) // P)) * P + (c % P)
    pn_T_cols = pn_hat[gmap]  # column c of device pn_T = graph gmap[c]
    for r in range(NCORES):
        lnc_dev = in_maps[r]["lncnt"].astype(np.float64)
        for j in range(JL):
            gb = r + 8 * j
            s_rows = np.sum([st[gb] for st in stage_s], axis=0)  # [128, D]
            p_rows = pn_full[gb * P : (gb + 1) * P]
            invs = 1.0 / np.sqrt(np.sum(s_rows**2, axis=1) + 1e-16)
            invp = 1.0 / np.sqrt(np.sum(p_rows**2, axis=1) + 1e-16)
            gram = s_rows @ pn_T_cols.T  # [128, 2048]
            simln = gram * invs[:, None] + lnc_dev[j]
            den = np.sum(np.exp(simln), axis=1)
            s0raw = np.sum(s_rows * (p_rows * invp[:, None]), axis=1)
            total += np.sum(np.log(den) - s0raw * invs)
    return total / G


if __name__ == "__main__":
    rng = np.random.default_rng(0)
    n = 200000
    inputs = {
        "logits_origin": rng.standard_normal((n, D), dtype=np.float32),
        "logits_pos": rng.standard_normal((n, D), dtype=np.float32),
        "ori_idx": rng.integers(0, G, n, dtype=np.int64),
        "pos_idx": rng.integers(0, G, n, dtype=np.int64),
        "neg_idx": rng.integers(0, G, (G, S), dtype=np.int64),
    }

    def np_ref(logits_origin, logits_pos, ori_idx, pos_idx, neg_idx):
        x = logits_origin.astype(np.float64)
        y = logits_pos.astype(np.float64)
        self_l = np.zeros((G, D))
        pos_l = np.zeros((G, D))
        np.add.at(self_l, ori_idx, x)
        np.add.at(pos_l, pos_idx, y)
        eps = 1e-8
        na = np.maximum(np.linalg.norm(self_l, axis=1), eps)
        nb = np.maximum(np.linalg.norm(pos_l, axis=1), eps)
        sh = self_l / na[:, None]
        ph = pos_l / nb[:, None]
        gram = sh @ ph.T
        sim0 = np.einsum("gd,gd->g", sh, ph)
        den = np.array(
            [np.exp(gram[g, neg_idx[g]]).sum() for g in range(G)]
        )
        res = np.log(den) - sim0
        return res.mean()

    expected = np_ref(**inputs)
    if os.environ.get("SELFTEST", "1") == "1":
        emu = _numpy_emulate(**inputs)
        err = abs(emu - expected) / max(abs(expected), 1e-12)
        print(f"emulate: expected={expected:.6f} emu={emu:.6f} relerr={err:.3e}")
    if os.environ.get("RUN_HW", "0") == "1":
        actual = kernel(**inputs)
        err = abs(actual - expected) / max(abs(expected), 1e-12)
        print(f"hw: expected={expected:.6f} actual={float(actual):.6f} relerr={err:.3e}")
